# revision 10
# baseline (speedup 1.0000x reference)
"""Trainium2 Bass kernel for Luong local-p sparse attention.

Math (per batch n, full shapes N=64, L=258, H=1024, Q=256):
    score = (h_t @ W_a) @ enc^T           masked to window [p_t-16, p_t+16]
    align = softmax(score) * gauss(p_t)
    out   = tanh([align @ enc, h_t] @ W_c^T)

Only a 33-wide window of enc survives the mask (window is +-16 around p_t),
so the kernel gathers windows host-side and pushes W_a / W_c[:, :H] through
the 33-wide side:
    u  = W_a-transform of window   (uT[h, (n,j)]  = sum_k W_aT[k,h] enc_w[(n,j),k])
    s  = uT^T-partial scores       (score^T[j, q] = sum_h uT[h,j] h_t[q,h])
    softmax over j (33 rows) done j-major with a 4th-power renormalization
    trick (no partition-dim max needed; partition sums via ones-matmul)
    v  = W_c1-transform of window  (v[(n,j), h']  = sum_h enc_w[(n,j),h] W_c1T[h,h'])
    out = tanh(h_t @ W_c2T + align^T.T @ v)

Data parallel over batch: 8 batches per core x 8 cores.  All matmuls run as
float32r (full-rate fp32 streaming mode).
"""

import numpy as np

import concourse.bass as bass
import concourse.bacc as bacc
import concourse.mybir as mybir
import concourse.tile as tile
from concourse.bass_utils import run_bass_kernel_spmd

# Problem constants (hardcoded per harness contract).
N, L, H, Q = 64, 258, 1024, 256
WINDOW = 16.0
DEV_POW = 128.0
NCORES = 8
B = N // NCORES  # batches per core
W = 33           # window width (positions that can survive the mask)
HC = H // 128    # h-chunks of 128 (PE contraction tiles)
F32 = mybir.dt.float32
F32R = mybir.dt.float32r
AF = mybir.ActivationFunctionType

# exp is computed as t = exp(s/4 + bias); bias = LOG_ALPHA keeps the
# column-sum T = sum_j t below fp32 max (t <= e^83, T <= 33*e^83 < e^88.7).
# The alpha scale cancels exactly in w = t/T.
LOG_ALPHA = -4.8520302  # -7*ln(2)
MASK_BIAS = -10000.0    # exp(<= -9900) == 0 in fp32


def build_nc() -> bass.Bass:
    nc = bacc.Bacc()
    enc_wT = nc.declare_dram_parameter("enc_wT", [H, B * W], F32R, isOutput=False)
    dec_hT = nc.declare_dram_parameter("dec_hT", [H, B * Q], F32R, isOutput=False)
    W_aT = nc.declare_dram_parameter("W_aT", [H, H], F32R, isOutput=False)
    W_c1T = nc.declare_dram_parameter("W_c1T", [H, H], F32R, isOutput=False)
    W_c2T = nc.declare_dram_parameter("W_c2T", [H, H], F32R, isOutput=False)
    biasT = nc.declare_dram_parameter("biasT", [W, B], F32, isOutput=False)
    gT = nc.declare_dram_parameter("gT", [W, B], F32, isOutput=False)
    onesD = nc.declare_dram_parameter("onesD", [W, W], F32R, isOutput=False)
    out = nc.declare_dram_parameter("out", [B * Q, H], F32, isOutput=True)

    with tile.TileContext(nc) as tc:
        with (
            tc.tile_pool(name="const", bufs=1) as cpool,
            tc.tile_pool(name="wstream", bufs=3) as wstream,
            tc.tile_pool(name="dec", bufs=3) as dec_pool,
            tc.tile_pool(name="sm", bufs=2) as sm_pool,
            tc.tile_pool(name="outp", bufs=3) as out_pool,
            tc.tile_pool(name="psA", bufs=4, space="PSUM") as psA,
            tc.tile_pool(name="psB", bufs=4, space="PSUM") as psB,
        ):
            # ---------------- resident tensors ----------------
            enc_sb = cpool.tile([128, HC, B * W], F32R)
            WaT_sb = cpool.tile([128, HC, H], F32R)
            Wc2T_sb = cpool.tile([128, HC, H], F32R)
            for kc in range(HC):
                nc.sync.dma_start(out=enc_sb[:, kc, :], in_=enc_wT[kc * 128:(kc + 1) * 128, :])
            for kc in range(HC):
                nc.sync.dma_start(out=WaT_sb[:, kc, :], in_=W_aT[kc * 128:(kc + 1) * 128, :])
            for kc in range(HC):
                nc.sync.dma_start(out=Wc2T_sb[:, kc, :], in_=W_c2T[kc * 128:(kc + 1) * 128, :])
            bias_sb = cpool.tile([W, B], F32)
            nc.sync.dma_start(out=bias_sb, in_=biasT[:, :])
            g_sb = cpool.tile([W, B], F32)
            nc.sync.dma_start(out=g_sb, in_=gT[:, :])
            ones_sb = cpool.tile([W, W], F32R)
            nc.sync.dma_start(out=ones_sb, in_=onesD[:, :])
            uT_sb = cpool.tile([128, HC, B * W], F32R)
            v_sb = cpool.tile([W, B, H], F32R)

            # ---------------- u phase: uT[h, (n,j)] ----------------
            # uT = W_aT.T @ enc_wT accumulated over k-chunks.
            for hc in range(HC):
                pu = psA.tile([128, B * W], F32, tag="A", name=f"pu{hc}")
                for kc in range(HC):
                    nc.tensor.matmul(
                        pu,
                        lhsT=WaT_sb[:, kc, hc * 128:(hc + 1) * 128],
                        rhs=enc_sb[:, kc, :],
                        start=(kc == 0),
                        stop=(kc == HC - 1),
                    )
                nc.scalar.copy(out=uT_sb[:, hc, :], in_=pu)

            # ---------------- v phase: v[(n,j), h'] ----------------
            # Batches packed 3-at-a-time into the 128-row M dim; W_c1T streamed.
            GROUPS = [(0, 99), (99, 99), (198, 66)]
            for nt in range(2):
                pvs = [psB.tile([128, 512], F32, tag="B", name=f"pv{nt}_{g}") for g in range(3)]
                for kc in range(HC):
                    wc1 = wstream.tile([128, 512], F32R, tag="wc1", name=f"wc1_{nt}_{kc}")
                    nc.gpsimd.dma_start(
                        out=wc1, in_=W_c1T[kc * 128:(kc + 1) * 128, nt * 512:(nt + 1) * 512]
                    )
                    for gi, (g0, glen) in enumerate(GROUPS):
                        nc.tensor.matmul(
                            pvs[gi][:glen, :],
                            lhsT=enc_sb[:, kc, g0:g0 + glen],
                            rhs=wc1[:],
                            start=(kc == 0),
                            stop=(kc == HC - 1),
                        )
                # Evacuate: DVE copy PSUM->SBUF (same partitions), then a
                # partition-reindexing SBUF->SBUF DMA per batch (row 33*b -> 0).
                for gi, (g0, glen) in enumerate(GROUPS):
                    vst = wstream.tile([128, 512], F32R, tag="vst", name=f"vst{nt}_{gi}")
                    nc.vector.tensor_copy(out=vst[:glen, :], in_=pvs[gi][:glen, :])
                    for off in range(glen // W):
                        n = gi * 3 + off
                        nc.sync.dma_start(
                            out=v_sb[:, n, nt * 512:(nt + 1) * 512],
                            in_=vst[off * W:(off + 1) * W, :],
                        )

            # ---------------- per-batch: score, softmax, output ----------------
            dec_r = dec_hT[:, :].rearrange("(c p) (n q) -> p c n q", p=128, q=Q)
            for n in range(B):
                dec_sb = dec_pool.tile([128, HC, Q], F32R, tag="dec", name=f"dec{n}")
                nc.sync.dma_start(out=dec_sb, in_=dec_r[:, :, n, :])

                # score^T[j, q] accumulated over h-chunks
                ps = psA.tile([W, Q], F32, tag="A", name=f"ps{n}")
                for hc in range(HC):
                    nc.tensor.matmul(
                        ps,
                        lhsT=uT_sb[:, hc, n * W:(n + 1) * W],
                        rhs=dec_sb[:, hc, :],
                        start=(hc == 0),
                        stop=(hc == HC - 1),
                    )

                # softmax over the 33 partitions, 4th-power renormalization:
                #   t = exp(s/4 + bias);  T = col-sum t (ones-matmul, result
                #   replicated to all 33 rows);  w = t/T;  w4 = w^4;
                #   Z = col-sum w4;  p = w4/Z = softmax(s);  align = p*g.
                t = sm_pool.tile([W, Q], F32R, tag="t", name=f"t{n}")
                nc.scalar.activation(
                    out=t, in_=ps, func=AF.Exp, bias=bias_sb[:, n:n + 1], scale=0.25
                )
                pT = psA.tile([W, Q], F32, tag="A", name=f"pT{n}")
                nc.tensor.matmul(
                    pT, lhsT=ones_sb[:], rhs=t[:],
                    start=True, stop=True,
                )
                rT = sm_pool.tile([W, Q], F32, tag="rT", name=f"rT{n}")
                nc.vector.reciprocal(out=rT, in_=pT)
                w = sm_pool.tile([W, Q], F32, tag="w", name=f"w{n}")
                nc.vector.tensor_mul(w, t, rT)
                w2 = sm_pool.tile([W, Q], F32, tag="w2", name=f"w2{n}")
                nc.vector.tensor_mul(w2, w, w)
                w4 = sm_pool.tile([W, Q], F32R, tag="w4", name=f"w4{n}")
                nc.vector.tensor_mul(w4, w2, w2)
                pZ = psA.tile([W, Q], F32, tag="A", name=f"pZ{n}")
                nc.tensor.matmul(
                    pZ, lhsT=ones_sb[:], rhs=w4[:],
                    start=True, stop=True,
                )
                rZ = sm_pool.tile([W, Q], F32, tag="rZ", name=f"rZ{n}")
                nc.vector.reciprocal(out=rZ, in_=pZ)
                p4 = sm_pool.tile([W, Q], F32, tag="p4", name=f"p4{n}")
                nc.vector.tensor_mul(p4, w4, rZ)
                align = sm_pool.tile([W, Q], F32R, tag="align", name=f"al{n}")
                nc.vector.tensor_scalar_mul(align, p4, g_sb[:, n:n + 1])

                # out[q, h'] = tanh(dec @ Wc2T + align^T.T @ v)
                for qt in range(2):
                    o_sb = out_pool.tile([128, H], F32, tag="o", name=f"o{n}_{qt}")
                    for ht in range(2):
                        po = psB.tile([128, 512], F32, tag="B", name=f"po{n}_{qt}_{ht}")
                        for hc in range(HC):
                            nc.tensor.matmul(
                                po,
                                lhsT=dec_sb[:, hc, qt * 128:(qt + 1) * 128],
                                rhs=Wc2T_sb[:, hc, ht * 512:(ht + 1) * 512],
                                start=(hc == 0),
                                stop=False,
                            )
                        nc.tensor.matmul(
                            po,
                            lhsT=align[:, qt * 128:(qt + 1) * 128],
                            rhs=v_sb[:, n, ht * 512:(ht + 1) * 512],
                            start=False,
                            stop=True,
                        )
                        nc.scalar.activation(
                            out=o_sb[:, ht * 512:(ht + 1) * 512], in_=po, func=AF.Tanh
                        )
                    nc.sync.dma_start(
                        out=out[n * Q + qt * 128:n * Q + (qt + 1) * 128, :], in_=o_sb
                    )
    nc.compile()
    return nc


def round_f32r(a: np.ndarray) -> np.ndarray:
    """Round fp32 to fp32r (TF32-like: 11-bit mantissa, low 12 bits zero),
    round-to-nearest-even.  This is what the PE consumes in fp32r mode."""
    u = np.ascontiguousarray(a, dtype=np.float32).view(np.uint32)
    lsb = (u >> np.uint32(12)) & np.uint32(1)
    u = (u + np.uint32(0x7FF) + lsb) & np.uint32(0xFFFFF000)
    return u.view(np.float32)


def prepare_in_maps(inputs: dict) -> list[dict]:
    enc = np.asarray(inputs["encoder_outputs"], dtype=np.float32)
    dec = np.asarray(inputs["decoder_h_t"], dtype=np.float32)
    src_len = np.asarray(inputs["src_len"], dtype=np.int32)
    p_t = np.asarray(inputs["p_t"], dtype=np.float32)
    W_a = np.asarray(inputs["W_a"], dtype=np.float32)
    W_c = np.asarray(inputs["W_c"], dtype=np.float32)

    # Window bounds, computed with the same fp32 ops as the reference.
    attn_start = np.maximum(p_t - np.float32(WINDOW), np.float32(0.0))
    attn_end = np.minimum(p_t + np.float32(WINDOW), src_len.astype(np.float32))
    s = np.ceil(attn_start).astype(np.int64)
    s = np.minimum(s, L - W)  # keep the 33-slice in bounds
    idx = s[:, None] + np.arange(W)[None, :]
    idxf = idx.astype(np.float32)
    mask = (idxf < attn_start[:, None]) | (idxf > attn_end[:, None])
    bias = np.where(mask, np.float32(MASK_BIAS), np.float32(LOG_ALPHA)).astype(np.float32)
    g = np.exp(-((idxf - p_t[:, None]) ** 2) / np.float32(DEV_POW)).astype(np.float32)

    enc_w = round_f32r(enc[np.arange(N)[:, None], idx, :])  # [N, W, H]
    dec = round_f32r(dec)
    W_aT = round_f32r(W_a.T)
    W_c1T = round_f32r(W_c[:, :H].T)
    W_c2T = round_f32r(W_c[:, H:].T)

    in_maps = []
    for c in range(NCORES):
        bs = slice(c * B, (c + 1) * B)
        in_maps.append({
            "enc_wT": np.ascontiguousarray(enc_w[bs].transpose(2, 0, 1).reshape(H, B * W)),
            "dec_hT": np.ascontiguousarray(dec[bs].transpose(2, 0, 1).reshape(H, B * Q)),
            "W_aT": W_aT,
            "W_c1T": W_c1T,
            "W_c2T": W_c2T,
            "biasT": np.ascontiguousarray(bias[bs].T),
            "onesD": np.ones((W, W), dtype=np.float32),
            "gT": np.ascontiguousarray(g[bs].T),
        })
    return in_maps


_NC = None


def get_nc() -> bass.Bass:
    global _NC
    if _NC is None:
        _NC = build_nc()
    return _NC


def kernel(**inputs) -> np.ndarray:
    nc = get_nc()
    in_maps = prepare_in_maps(inputs)
    res = run_bass_kernel_spmd(nc, in_maps, list(range(NCORES)))
    outs = [res.results[c]["out"].reshape(B, Q, H) for c in range(NCORES)]
    return np.concatenate(outs, axis=0)


# revision 18
# speedup vs baseline: 26.8611x; 26.8611x over previous
"""Trainium2 Bass kernel for Luong local-p sparse attention.

Math (per batch n, full shapes N=64, L=258, H=1024, Q=256):
    score = (h_t @ W_a) @ enc^T           masked to window [p_t-16, p_t+16]
    align = softmax(score) * gauss(p_t)
    out   = tanh([align @ enc, h_t] @ W_c^T)

Only a 33-wide window of enc survives the mask (window is +-16 around p_t),
so the kernel gathers windows host-side and pushes W_a / W_c[:, :H] through
the 33-wide side:
    u  = W_a-transform of window   (uT[h, (n,j)]  = sum_k W_aT[k,h] enc_w[(n,j),k])
    s  = uT^T-partial scores       (score^T[j, q] = sum_h uT[h,j] h_t[q,h])
    softmax over j (33 rows) done j-major with a 4th-power renormalization
    trick (no partition-dim max needed; partition sums via ones-matmul)
    v  = W_c1-transform of window  (v[(n,j), h']  = sum_h enc_w[(n,j),h] W_c1T[h,h'])
    out = tanh(h_t @ W_c2T + align^T.T @ v)

Data parallel over batch: 8 batches per core x 8 cores.  All matmuls run as
float32r (full-rate fp32 streaming mode).
"""

import numpy as np

import concourse.bass as bass
import concourse.bacc as bacc
import concourse.mybir as mybir
import concourse.tile as tile
from concourse.bass_utils import run_bass_kernel_spmd

# Problem constants (hardcoded per harness contract).
N, L, H, Q = 64, 258, 1024, 256
WINDOW = 16.0
DEV_POW = 128.0
NCORES = 8
B = N // NCORES  # batches per core
W = 33           # window width (positions that can survive the mask)
HC = H // 128    # h-chunks of 128 (PE contraction tiles)
F32 = mybir.dt.float32
F32R = mybir.dt.float32r
AF = mybir.ActivationFunctionType

# exp is computed as t = exp(s/4 + bias); bias = LOG_ALPHA keeps the
# column-sum T = sum_j t below fp32 max (t <= e^83, T <= 33*e^83 < e^88.7).
# The alpha scale cancels exactly in w = t/T.
LOG_ALPHA = -4.8520302  # -7*ln(2)
MASK_BIAS = -10000.0    # exp(<= -9900) == 0 in fp32


def build_nc() -> bass.Bass:
    nc = bacc.Bacc()
    enc_wT = nc.declare_dram_parameter("enc_wT", [H, B * W], F32R, isOutput=False)
    dec_hT = nc.declare_dram_parameter("dec_hT", [H, B * Q], F32R, isOutput=False)
    W_aT = nc.declare_dram_parameter("W_aT", [H, H], F32R, isOutput=False)
    W_c1T = nc.declare_dram_parameter("W_c1T", [H, H], F32R, isOutput=False)
    W_c2T = nc.declare_dram_parameter("W_c2T", [H, H], F32R, isOutput=False)
    biasT = nc.declare_dram_parameter("biasT", [W, B], F32, isOutput=False)
    gT = nc.declare_dram_parameter("gT", [W, B], F32, isOutput=False)
    onesD = nc.declare_dram_parameter("onesD", [W, W], F32R, isOutput=False)
    gPackT = nc.declare_dram_parameter("gPackT", [3 * W, 3], F32, isOutput=False)
    out = nc.declare_dram_parameter("out", [B * Q, H], F32, isOutput=True)

    with tile.TileContext(nc) as tc:
        with (
            tc.tile_pool(name="const", bufs=1) as cpool,
            tc.tile_pool(name="wc1p", bufs=8) as wc1p,
            tc.tile_pool(name="vstp", bufs=3) as vstp,
            tc.tile_pool(name="dec", bufs=2) as dec_pool,
            tc.tile_pool(name="sm", bufs=2) as sm_pool,
            tc.tile_pool(name="outp", bufs=2) as out_pool,
            tc.tile_pool(name="psA", bufs=2, space="PSUM") as psA,
            tc.tile_pool(name="psB", bufs=6, space="PSUM") as psB,
        ):
            # ---------------- resident tensors ----------------
            enc_sb = cpool.tile([128, HC, B * W], F32R)
            WaT_sb = cpool.tile([128, HC, H], F32R)
            Wc2T_sb = cpool.tile([128, HC, H], F32R)
            bias_sb = cpool.tile([W, B], F32)
            g_sb = cpool.tile([W, B], F32)
            gpack_sb = cpool.tile([3 * W, 3], F32)
            ones_sb = cpool.tile([W, W], F32R)
            uT_sb = cpool.tile([128, HC, B * W], F32R)
            v_sb = cpool.tile([W, B, H], F32R)

            enc_r = enc_wT[:, :].rearrange("(c p) m -> p c m", p=128)
            WaT_r = W_aT[:, :].rearrange("(c p) m -> p c m", p=128)
            Wc2_r = W_c2T[:, :].rearrange("(c p) m -> p c m", p=128)
            Wc1_r = W_c1T[:, :].rearrange("(cp p) m -> p cp m", p=128)
            dec_r = dec_hT[:, :].rearrange("(c p) (n q) -> p c n q", p=128, q=Q)

            # DMA issue order is the schedule.  sync ring is ~1.5x faster:
            # it carries enc + most of W_aT + all of W_c2T; scalar carries
            # the rest of W_aT + most of W_c1T + early dec batches.
            nc.sync.dma_start(out=enc_sb, in_=enc_r)
            nc.sync.dma_start(out=WaT_sb[:, 0:2, :], in_=WaT_r[:, 0:2, :])
            nc.scalar.dma_start(out=WaT_sb[:, 4:6, :], in_=WaT_r[:, 4:6, :])
            nc.sync.dma_start(out=WaT_sb[:, 2:4, :], in_=WaT_r[:, 2:4, :])
            nc.scalar.dma_start(out=WaT_sb[:, 6:8, :], in_=WaT_r[:, 6:8, :])
            nc.scalar.dma_start(out=bias_sb, in_=biasT[:, :])
            nc.scalar.dma_start(out=g_sb, in_=gT[:, :])
            nc.scalar.dma_start(out=gpack_sb, in_=gPackT[:, :])
            nc.scalar.dma_start(out=ones_sb, in_=onesD[:, :])

            dec_tiles = {}
            for n in range(2):
                dt_ = dec_pool.tile([128, HC, Q], F32R, tag="dec", name=f"dec{n}")
                eng = nc.sync if n % 2 == 0 else nc.scalar
                eng.dma_start(out=dt_, in_=dec_r[:, :, n, :])
                dec_tiles[n] = dt_

            # W_c2T on the fast ring (batch 0's dec_out is paced by it).
            for i in range(2):
                nc.sync.dma_start(out=Wc2T_sb[:, 4 * i:4 * i + 4, :], in_=Wc2_r[:, 4 * i:4 * i + 4, :])

            # All W_c1T chunks resident (consumed by the v phase), mostly on
            # the scalar ring.
            wc1_tiles = {}
            for nt in range(2):
                for kcp in range(4):
                    wt = wc1p.tile([128, 2, 512], F32R, tag="wc1", name=f"wc1_{nt}_{kcp}")
                    eng = nc.sync if kcp == 3 else nc.scalar
                    eng.dma_start(
                        out=wt,
                        in_=Wc1_r[:, 2 * kcp:2 * kcp + 2, nt * 512:(nt + 1) * 512],
                    )
                    wc1_tiles[(nt, kcp)] = wt

            # ---------------- u phase: uT[h, (n,j)] ----------------
            for hc in range(HC):
                pu = psB.tile([128, B * W], F32, tag="B", name=f"pu{hc}")
                for kc in range(HC):
                    nc.tensor.matmul(
                        pu,
                        lhsT=WaT_sb[:, kc, hc * 128:(hc + 1) * 128],
                        rhs=enc_sb[:, kc, :],
                        start=(kc == 0),
                        stop=(kc == HC - 1),
                    )
                nc.scalar.copy(out=uT_sb[:, hc, :], in_=pu)

            # ---------------- v phase helpers (emitted between batches) ----
            GROUPS = [(0, 99), (99, 99), (198, 66)]

            def v_group(gi):
                g0, glen = GROUPS[gi]
                for nt in range(2):
                    pv = psB.tile([128, 512], F32, tag="B", name=f"pv{nt}_{gi}")
                    for kcp in range(4):
                        for j in range(2):
                            kc = 2 * kcp + j
                            nc.tensor.matmul(
                                pv[:glen, :],
                                lhsT=enc_sb[:, kc, g0:g0 + glen],
                                rhs=wc1_tiles[(nt, kcp)][:, j, :],
                                start=(kc == 0),
                                stop=(kc == HC - 1),
                            )
                    vst = vstp.tile([128, 512], F32R, tag="vst", name=f"vst{nt}_{gi}")
                    # evacuate + fold the gaussian in one op: ctx = p4 @ (g*v)
                    nc.vector.tensor_scalar_mul(
                        vst[:glen, :], pv[:glen, :], gpack_sb[:glen, gi:gi + 1]
                    )
                    for off in range(glen // W):
                        n = gi * 3 + off
                        eng = nc.sync if (off + nt) % 2 == 0 else nc.scalar
                        eng.dma_start(
                            out=v_sb[:, n, nt * 512:(nt + 1) * 512],
                            in_=vst[off * W:(off + 1) * W, :],
                        )

            # ---------------- per-batch emission ----------------
            prev = None  # (n, pos, o_sb) awaiting tanh + store

            def flush_prev(split_store=False):
                nonlocal prev
                if prev is None:
                    return
                pn, ppos, po_sb = prev
                for qt in range(2):
                    for ht in range(2):
                        nc.scalar.activation(
                            out=po_sb[:, qt, ht * 512:(ht + 1) * 512],
                            in_=ppos[(qt, ht)], func=AF.Tanh,
                        )
                dst = out[pn * Q:(pn + 1) * Q, :].rearrange("(qt p) h -> p qt h", p=128)
                if split_store:
                    nc.sync.dma_start(out=dst[:, 0, :], in_=po_sb[:, 0, :])
                    nc.scalar.dma_start(out=dst[:, 1, :], in_=po_sb[:, 1, :])
                else:
                    eng = nc.sync if pn % 2 == 0 else nc.scalar
                    eng.dma_start(out=dst, in_=po_sb)
                prev = None

            state = {}

            def batch_pre(n):
                if n in dec_tiles:
                    dec_sb = dec_tiles[n]
                else:
                    dec_sb = dec_pool.tile([128, HC, Q], F32R, tag="dec", name=f"dec{n}")
                    eng = nc.sync if n % 2 == 0 else nc.scalar
                    eng.dma_start(out=dec_sb, in_=dec_r[:, :, n, :])

                ps = psA.tile([W, Q], F32, tag="A", name=f"ps{n}")
                for hc in range(HC):
                    nc.tensor.matmul(
                        ps,
                        lhsT=uT_sb[:, hc, n * W:(n + 1) * W],
                        rhs=dec_sb[:, hc, :],
                        start=(hc == 0),
                        stop=(hc == HC - 1),
                    )
                # softmax over 33 partitions via 4th-power renormalization,
                # in place in one tile: t=exp(s/4+b); T=colsum t; t=(t/T)^4;
                # Z=colsum t; t/=Z (-> p4).  PE bits hide in the dec stream.
                t = sm_pool.tile([W, Q], F32R, tag="t", name=f"t{n}")
                nc.scalar.activation(
                    out=t, in_=ps, func=AF.Exp, bias=bias_sb[:, n:n + 1], scale=0.25
                )
                flush_prev()
                o_sb = out_pool.tile([128, 2, H], F32, tag="o", name=f"o{n}")
                pos = {}

                def dec_group(qt, ht):
                    po = psB.tile([128, 512], F32, tag="B", name=f"po{n}_{qt}_{ht}")
                    pos[(qt, ht)] = po
                    for hc in range(HC):
                        nc.tensor.matmul(
                            po,
                            lhsT=dec_sb[:, hc, qt * 128:(qt + 1) * 128],
                            rhs=Wc2T_sb[:, hc, ht * 512:(ht + 1) * 512],
                            start=(hc == 0),
                            stop=False,
                        )

                dec_group(0, 0)
                pT = psA.tile([W, Q], F32, tag="A", name=f"pT{n}")
                nc.tensor.matmul(pT, lhsT=ones_sb[:], rhs=t[:], start=True, stop=True)
                rT = sm_pool.tile([W, Q], F32, tag="rT", name=f"rT{n}")
                nc.vector.reciprocal(out=rT, in_=pT)
                nc.vector.tensor_mul(t, t, rT)
                nc.vector.tensor_mul(t, t, t)
                nc.vector.tensor_mul(t, t, t)
                dec_group(0, 1)
                dec_group(1, 0)
                pZ = psA.tile([W, Q], F32, tag="A", name=f"pZ{n}")
                nc.tensor.matmul(pZ, lhsT=ones_sb[:], rhs=t[:], start=True, stop=True)
                rZ = sm_pool.tile([W, Q], F32, tag="rZ", name=f"rZ{n}")
                nc.vector.reciprocal(out=rZ, in_=pZ)
                nc.vector.tensor_mul(t, t, rZ)
                dec_group(1, 1)
                state[n] = (t, pos, o_sb)

            def batch_ctx(n):
                t, pos, o_sb = state.pop(n)
                for qt in range(2):
                    for ht in range(2):
                        nc.tensor.matmul(
                            pos[(qt, ht)],
                            lhsT=t[:, qt * 128:(qt + 1) * 128],
                            rhs=v_sb[:, n, ht * 512:(ht + 1) * 512],
                            start=False,
                            stop=True,
                        )
                nonlocal prev
                prev = (n, pos, o_sb)

            batch_pre(0)
            v_group(0)
            batch_ctx(0)
            batch_pre(1)
            batch_ctx(1)
            batch_pre(2)
            batch_ctx(2)
            v_group(1)
            batch_pre(3)
            batch_ctx(3)
            batch_pre(4)
            batch_ctx(4)
            v_group(2)
            for n in range(5, B):
                batch_pre(n)
                batch_ctx(n)
            flush_prev(split_store=True)
    nc.compile()
    return nc


def round_f32r(a: np.ndarray) -> np.ndarray:
    """Round fp32 to fp32r (TF32-like: 11-bit mantissa, low 12 bits zero),
    round-to-nearest-even.  This is what the PE consumes in fp32r mode."""
    u = np.ascontiguousarray(a, dtype=np.float32).view(np.uint32)
    lsb = (u >> np.uint32(12)) & np.uint32(1)
    u = (u + np.uint32(0x7FF) + lsb) & np.uint32(0xFFFFF000)
    return u.view(np.float32)


def prepare_in_maps(inputs: dict) -> list[dict]:
    enc = np.asarray(inputs["encoder_outputs"], dtype=np.float32)
    dec = np.asarray(inputs["decoder_h_t"], dtype=np.float32)
    src_len = np.asarray(inputs["src_len"], dtype=np.int32)
    p_t = np.asarray(inputs["p_t"], dtype=np.float32)
    W_a = np.asarray(inputs["W_a"], dtype=np.float32)
    W_c = np.asarray(inputs["W_c"], dtype=np.float32)

    # Window bounds, computed with the same fp32 ops as the reference.
    attn_start = np.maximum(p_t - np.float32(WINDOW), np.float32(0.0))
    attn_end = np.minimum(p_t + np.float32(WINDOW), src_len.astype(np.float32))
    s = np.ceil(attn_start).astype(np.int64)
    s = np.minimum(s, L - W)  # keep the 33-slice in bounds
    idx = s[:, None] + np.arange(W)[None, :]
    idxf = idx.astype(np.float32)
    mask = (idxf < attn_start[:, None]) | (idxf > attn_end[:, None])
    bias = np.where(mask, np.float32(MASK_BIAS), np.float32(LOG_ALPHA)).astype(np.float32)
    g = np.exp(-((idxf - p_t[:, None]) ** 2) / np.float32(DEV_POW)).astype(np.float32)

    enc_w = round_f32r(enc[np.arange(N)[:, None], idx, :])  # [N, W, H]
    dec = round_f32r(dec)
    W_aT = round_f32r(W_a.T)
    W_c1T = round_f32r(W_c[:, :H].T)
    W_c2T = round_f32r(W_c[:, H:].T)

    in_maps = []
    for c in range(NCORES):
        bs = slice(c * B, (c + 1) * B)
        gc = g[bs]  # [B, W]
        gpack = np.zeros((3 * W, 3), dtype=np.float32)
        for n in range(B):
            gi, off = divmod(n, 3)
            gpack[off * W:(off + 1) * W, gi] = gc[n]
        in_maps.append({
            "enc_wT": np.ascontiguousarray(enc_w[bs].transpose(2, 0, 1).reshape(H, B * W)),
            "dec_hT": np.ascontiguousarray(dec[bs].transpose(2, 0, 1).reshape(H, B * Q)),
            "W_aT": W_aT,
            "W_c1T": W_c1T,
            "W_c2T": W_c2T,
            "biasT": np.ascontiguousarray(bias[bs].T),
            "onesD": np.ones((W, W), dtype=np.float32),
            "gPackT": gpack,
            "gT": np.ascontiguousarray(g[bs].T),
        })
    return in_maps


_NC = None


def get_nc() -> bass.Bass:
    global _NC
    if _NC is None:
        _NC = build_nc()
    return _NC


def kernel(**inputs) -> np.ndarray:
    nc = get_nc()
    in_maps = prepare_in_maps(inputs)
    res = run_bass_kernel_spmd(nc, in_maps, list(range(NCORES)))
    outs = [res.results[c]["out"].reshape(B, Q, H) for c in range(NCORES)]
    return np.concatenate(outs, axis=0)


# revision 20
# speedup vs baseline: 26.9108x; 1.0018x over previous
"""Trainium2 Bass kernel for Luong local-p sparse attention.

Math (per batch n, full shapes N=64, L=258, H=1024, Q=256):
    score = (h_t @ W_a) @ enc^T           masked to window [p_t-16, p_t+16]
    align = softmax(score) * gauss(p_t)
    out   = tanh([align @ enc, h_t] @ W_c^T)

Only a 33-wide window of enc survives the mask (window is +-16 around p_t),
so the kernel gathers windows host-side and pushes W_a / W_c[:, :H] through
the 33-wide side:
    u  = W_a-transform of window   (uT[h, (n,j)]  = sum_k W_aT[k,h] enc_w[(n,j),k])
    s  = uT^T-partial scores       (score^T[j, q] = sum_h uT[h,j] h_t[q,h])
    softmax over j (33 rows) done j-major with a 4th-power renormalization
    trick (no partition-dim max needed; partition sums via ones-matmul)
    v  = W_c1-transform of window  (v[(n,j), h']  = sum_h enc_w[(n,j),h] W_c1T[h,h'])
    out = tanh(h_t @ W_c2T + align^T.T @ v)

Data parallel over batch: 8 batches per core x 8 cores.  All matmuls run as
float32r (full-rate fp32 streaming mode).
"""

import numpy as np

import concourse.bass as bass
import concourse.bacc as bacc
import concourse.mybir as mybir
import concourse.tile as tile
from concourse.bass_utils import run_bass_kernel_spmd

# Problem constants (hardcoded per harness contract).
N, L, H, Q = 64, 258, 1024, 256
WINDOW = 16.0
DEV_POW = 128.0
NCORES = 8
B = N // NCORES  # batches per core
W = 33           # window width (positions that can survive the mask)
HC = H // 128    # h-chunks of 128 (PE contraction tiles)
F32 = mybir.dt.float32
F32R = mybir.dt.float32r
AF = mybir.ActivationFunctionType

# exp is computed as t = exp(s/4 + bias); bias = LOG_ALPHA keeps the
# column-sum T = sum_j t below fp32 max (t <= e^83, T <= 33*e^83 < e^88.7).
# The alpha scale cancels exactly in w = t/T.
LOG_ALPHA = -4.8520302  # -7*ln(2)
MASK_BIAS = -10000.0    # exp(<= -9900) == 0 in fp32


def build_nc() -> bass.Bass:
    nc = bacc.Bacc()
    enc_wT = nc.declare_dram_parameter("enc_wT", [H, B * W], F32R, isOutput=False)
    dec_hT = nc.declare_dram_parameter("dec_hT", [H, B * Q], F32R, isOutput=False)
    W_aT = nc.declare_dram_parameter("W_aT", [H, H], F32R, isOutput=False)
    W_c1T = nc.declare_dram_parameter("W_c1T", [H, H], F32R, isOutput=False)
    W_c2T = nc.declare_dram_parameter("W_c2T", [H, H], F32R, isOutput=False)
    biasT = nc.declare_dram_parameter("biasT", [W, B], F32, isOutput=False)
    gT = nc.declare_dram_parameter("gT", [W, B], F32, isOutput=False)
    onesD = nc.declare_dram_parameter("onesD", [W, W], F32R, isOutput=False)
    gPackT = nc.declare_dram_parameter("gPackT", [3 * W, 3], F32, isOutput=False)
    out = nc.declare_dram_parameter("out", [B * Q, H], F32, isOutput=True)

    with tile.TileContext(nc) as tc:
        with (
            tc.tile_pool(name="const", bufs=1) as cpool,
            tc.tile_pool(name="wc1p", bufs=8) as wc1p,
            tc.tile_pool(name="vstp", bufs=3) as vstp,
            tc.tile_pool(name="dec", bufs=2) as dec_pool,
            tc.tile_pool(name="sm", bufs=2) as sm_pool,
            tc.tile_pool(name="outp", bufs=2) as out_pool,
            tc.tile_pool(name="psA", bufs=2, space="PSUM") as psA,
            tc.tile_pool(name="psB", bufs=6, space="PSUM") as psB,
        ):
            # ---------------- resident tensors ----------------
            enc_sb = cpool.tile([128, HC, B * W], F32R)
            WaT_sb = cpool.tile([128, HC, H], F32R)
            Wc2T_sb = cpool.tile([128, HC, H], F32R)
            bias_sb = cpool.tile([W, B], F32)
            g_sb = cpool.tile([W, B], F32)
            gpack_sb = cpool.tile([3 * W, 3], F32)
            ones_sb = cpool.tile([W, W], F32R)
            uT_sb = cpool.tile([128, HC, B * W], F32R)
            v_sb = cpool.tile([W, B, H], F32R)

            enc_r = enc_wT[:, :].rearrange("(c p) m -> p c m", p=128)
            WaT_r = W_aT[:, :].rearrange("(c p) m -> p c m", p=128)
            Wc2_r = W_c2T[:, :].rearrange("(c p) m -> p c m", p=128)
            Wc1_r = W_c1T[:, :].rearrange("(cp p) m -> p cp m", p=128)
            dec_r = dec_hT[:, :].rearrange("(c p) (n q) -> p c n q", p=128, q=Q)

            # DMA issue order is the schedule.  sync ring is ~1.5x faster:
            # it carries enc + most of W_aT + all of W_c2T; scalar carries
            # the rest of W_aT + most of W_c1T + early dec batches.
            nc.sync.dma_start(out=enc_sb, in_=enc_r)
            nc.sync.dma_start(out=WaT_sb[:, 0:2, :], in_=WaT_r[:, 0:2, :])
            nc.scalar.dma_start(out=WaT_sb[:, 4:6, :], in_=WaT_r[:, 4:6, :])
            nc.sync.dma_start(out=WaT_sb[:, 2:4, :], in_=WaT_r[:, 2:4, :])
            nc.scalar.dma_start(out=WaT_sb[:, 6:8, :], in_=WaT_r[:, 6:8, :])
            nc.scalar.dma_start(out=bias_sb, in_=biasT[:, :])
            nc.scalar.dma_start(out=g_sb, in_=gT[:, :])
            nc.scalar.dma_start(out=gpack_sb, in_=gPackT[:, :])
            nc.scalar.dma_start(out=ones_sb, in_=onesD[:, :])

            dec_tiles = {}
            for n in range(2):
                dt_ = dec_pool.tile([128, HC, Q], F32R, tag="dec", name=f"dec{n}")
                eng = nc.sync if n % 2 == 0 else nc.scalar
                eng.dma_start(out=dt_, in_=dec_r[:, :, n, :])
                dec_tiles[n] = dt_

            # W_c2T on the fast ring (batch 0's dec_out is paced by it).
            for i in range(2):
                nc.sync.dma_start(out=Wc2T_sb[:, 4 * i:4 * i + 4, :], in_=Wc2_r[:, 4 * i:4 * i + 4, :])

            # All W_c1T chunks resident (consumed by the v phase), mostly on
            # the scalar ring.
            wc1_tiles = {}
            for nt in range(2):
                for kcp in range(4):
                    wt = wc1p.tile([128, 2, 512], F32R, tag="wc1", name=f"wc1_{nt}_{kcp}")
                    eng = nc.sync if kcp == 3 else nc.scalar
                    eng.dma_start(
                        out=wt,
                        in_=Wc1_r[:, 2 * kcp:2 * kcp + 2, nt * 512:(nt + 1) * 512],
                    )
                    wc1_tiles[(nt, kcp)] = wt

            # ---------------- u phase: uT[h, (n,j)] ----------------
            for hc in range(HC):
                pu = psB.tile([128, B * W], F32, tag="B", name=f"pu{hc}")
                for kc in range(HC):
                    nc.tensor.matmul(
                        pu,
                        lhsT=WaT_sb[:, kc, hc * 128:(hc + 1) * 128],
                        rhs=enc_sb[:, kc, :],
                        start=(kc == 0),
                        stop=(kc == HC - 1),
                    )
                nc.scalar.copy(out=uT_sb[:, hc, :], in_=pu)

            # ---------------- v phase helpers (emitted between batches) ----
            GROUPS = [(0, 99), (99, 99), (198, 66)]

            def v_group(gi):
                g0, glen = GROUPS[gi]
                for nt in range(2):
                    pv = psB.tile([128, 512], F32, tag="B", name=f"pv{nt}_{gi}")
                    for kcp in range(4):
                        for j in range(2):
                            kc = 2 * kcp + j
                            nc.tensor.matmul(
                                pv[:glen, :],
                                lhsT=enc_sb[:, kc, g0:g0 + glen],
                                rhs=wc1_tiles[(nt, kcp)][:, j, :],
                                start=(kc == 0),
                                stop=(kc == HC - 1),
                            )
                    vst = vstp.tile([128, 512], F32R, tag="vst", name=f"vst{nt}_{gi}")
                    # evacuate + fold the gaussian in one op: ctx = p4 @ (g*v)
                    nc.vector.tensor_scalar_mul(
                        vst[:glen, :], pv[:glen, :], gpack_sb[:glen, gi:gi + 1]
                    )
                    for off in range(glen // W):
                        n = gi * 3 + off
                        eng = nc.sync if (off + nt) % 2 == 0 else nc.scalar
                        eng.dma_start(
                            out=v_sb[:, n, nt * 512:(nt + 1) * 512],
                            in_=vst[off * W:(off + 1) * W, :],
                        )

            # ---------------- per-batch emission ----------------
            prev = None  # (n, pos, o_sb) awaiting tanh + store

            def flush_prev(split_store=False):
                nonlocal prev
                if prev is None:
                    return
                pn, ppos, po_sb = prev
                for qt in range(2):
                    for ht in range(2):
                        nc.scalar.activation(
                            out=po_sb[:, qt, ht * 512:(ht + 1) * 512],
                            in_=ppos[(qt, ht)], func=AF.Tanh,
                        )
                dst = out[pn * Q:(pn + 1) * Q, :].rearrange("(qt p) h -> p qt h", p=128)
                if split_store:
                    nc.sync.dma_start(out=dst[:, 0, :], in_=po_sb[:, 0, :])
                    nc.scalar.dma_start(out=dst[:, 1, :], in_=po_sb[:, 1, :])
                else:
                    eng = nc.sync if pn % 2 == 0 else nc.scalar
                    eng.dma_start(out=dst, in_=po_sb)
                prev = None

            state = {}
            scored = {}

            def score_part(n):
                if n in dec_tiles:
                    dec_sb = dec_tiles[n]
                else:
                    dec_sb = dec_pool.tile([128, HC, Q], F32R, tag="dec", name=f"dec{n}")
                    eng = nc.sync if n % 2 == 0 else nc.scalar
                    eng.dma_start(out=dec_sb, in_=dec_r[:, :, n, :])

                ps = psA.tile([W, Q], F32, tag="A", name=f"ps{n}")
                for hc in range(HC):
                    nc.tensor.matmul(
                        ps,
                        lhsT=uT_sb[:, hc, n * W:(n + 1) * W],
                        rhs=dec_sb[:, hc, :],
                        start=(hc == 0),
                        stop=(hc == HC - 1),
                    )
                # softmax over 33 partitions via 4th-power renormalization,
                # in place in one tile: t=exp(s/4+b); T=colsum t; t=(t/T)^4;
                # Z=colsum t; t/=Z (-> p4).  PE bits hide in the dec stream.
                t = sm_pool.tile([W, Q], F32R, tag="t", name=f"t{n}")
                nc.scalar.activation(
                    out=t, in_=ps, func=AF.Exp, bias=bias_sb[:, n:n + 1], scale=0.25
                )
                scored[n] = (dec_sb, t)

            def batch_pre(n):
                if n not in scored:
                    score_part(n)
                dec_sb, t = scored.pop(n)
                flush_prev()
                o_sb = out_pool.tile([128, 2, H], F32, tag="o", name=f"o{n}")
                pos = {}

                def dec_group(qt, ht):
                    po = psB.tile([128, 512], F32, tag="B", name=f"po{n}_{qt}_{ht}")
                    pos[(qt, ht)] = po
                    for hc in range(HC):
                        nc.tensor.matmul(
                            po,
                            lhsT=dec_sb[:, hc, qt * 128:(qt + 1) * 128],
                            rhs=Wc2T_sb[:, hc, ht * 512:(ht + 1) * 512],
                            start=(hc == 0),
                            stop=False,
                        )

                dec_group(0, 0)
                pT = psA.tile([W, Q], F32, tag="A", name=f"pT{n}")
                nc.tensor.matmul(pT, lhsT=ones_sb[:], rhs=t[:], start=True, stop=True)
                rT = sm_pool.tile([W, Q], F32, tag="rT", name=f"rT{n}")
                nc.vector.reciprocal(out=rT, in_=pT)
                nc.vector.tensor_mul(t, t, rT)
                nc.vector.tensor_mul(t, t, t)
                nc.vector.tensor_mul(t, t, t)
                dec_group(0, 1)
                dec_group(1, 0)
                pZ = psA.tile([W, Q], F32, tag="A", name=f"pZ{n}")
                nc.tensor.matmul(pZ, lhsT=ones_sb[:], rhs=t[:], start=True, stop=True)
                rZ = sm_pool.tile([W, Q], F32, tag="rZ", name=f"rZ{n}")
                nc.vector.reciprocal(out=rZ, in_=pZ)
                nc.vector.tensor_mul(t, t, rZ)
                dec_group(1, 1)
                state[n] = (t, pos, o_sb)

            def batch_ctx(n):
                t, pos, o_sb = state.pop(n)
                for qt in range(2):
                    for ht in range(2):
                        nc.tensor.matmul(
                            pos[(qt, ht)],
                            lhsT=t[:, qt * 128:(qt + 1) * 128],
                            rhs=v_sb[:, n, ht * 512:(ht + 1) * 512],
                            start=False,
                            stop=True,
                        )
                nonlocal prev
                prev = (n, pos, o_sb)

            batch_pre(0)
            v_group(0)
            batch_ctx(0)
            batch_pre(1)
            batch_ctx(1)
            batch_pre(2)
            batch_ctx(2)
            v_group(1)
            batch_pre(3)
            batch_ctx(3)
            batch_pre(4)
            batch_ctx(4)
            v_group(2)
            for n in range(5, B):
                batch_pre(n)
                batch_ctx(n)
            flush_prev(split_store=True)
    nc.compile()
    return nc


def round_f32r(a: np.ndarray) -> np.ndarray:
    """Round fp32 to fp32r (TF32-like: 11-bit mantissa, low 12 bits zero),
    round-to-nearest-even.  This is what the PE consumes in fp32r mode."""
    u = np.ascontiguousarray(a, dtype=np.float32).view(np.uint32)
    lsb = (u >> np.uint32(12)) & np.uint32(1)
    u = (u + np.uint32(0x7FF) + lsb) & np.uint32(0xFFFFF000)
    return u.view(np.float32)


def prepare_in_maps(inputs: dict) -> list[dict]:
    enc = np.asarray(inputs["encoder_outputs"], dtype=np.float32)
    dec = np.asarray(inputs["decoder_h_t"], dtype=np.float32)
    src_len = np.asarray(inputs["src_len"], dtype=np.int32)
    p_t = np.asarray(inputs["p_t"], dtype=np.float32)
    W_a = np.asarray(inputs["W_a"], dtype=np.float32)
    W_c = np.asarray(inputs["W_c"], dtype=np.float32)

    # Window bounds, computed with the same fp32 ops as the reference.
    attn_start = np.maximum(p_t - np.float32(WINDOW), np.float32(0.0))
    attn_end = np.minimum(p_t + np.float32(WINDOW), src_len.astype(np.float32))
    s = np.ceil(attn_start).astype(np.int64)
    s = np.minimum(s, L - W)  # keep the 33-slice in bounds
    idx = s[:, None] + np.arange(W)[None, :]
    idxf = idx.astype(np.float32)
    mask = (idxf < attn_start[:, None]) | (idxf > attn_end[:, None])
    bias = np.where(mask, np.float32(MASK_BIAS), np.float32(LOG_ALPHA)).astype(np.float32)
    g = np.exp(-((idxf - p_t[:, None]) ** 2) / np.float32(DEV_POW)).astype(np.float32)

    enc_w = round_f32r(enc[np.arange(N)[:, None], idx, :])  # [N, W, H]
    dec = round_f32r(dec)
    W_aT = round_f32r(W_a.T)
    W_c1T = round_f32r(W_c[:, :H].T)
    W_c2T = round_f32r(W_c[:, H:].T)

    in_maps = []
    for c in range(NCORES):
        bs = slice(c * B, (c + 1) * B)
        gc = g[bs]  # [B, W]
        gpack = np.zeros((3 * W, 3), dtype=np.float32)
        for n in range(B):
            gi, off = divmod(n, 3)
            gpack[off * W:(off + 1) * W, gi] = gc[n]
        in_maps.append({
            "enc_wT": np.ascontiguousarray(enc_w[bs].transpose(2, 0, 1).reshape(H, B * W)),
            "dec_hT": np.ascontiguousarray(dec[bs].transpose(2, 0, 1).reshape(H, B * Q)),
            "W_aT": W_aT,
            "W_c1T": W_c1T,
            "W_c2T": W_c2T,
            "biasT": np.ascontiguousarray(bias[bs].T),
            "onesD": np.ones((W, W), dtype=np.float32),
            "gPackT": gpack,
            "gT": np.ascontiguousarray(g[bs].T),
        })
    return in_maps


_NC = None


def get_nc() -> bass.Bass:
    global _NC
    if _NC is None:
        _NC = build_nc()
    return _NC


def kernel(**inputs) -> np.ndarray:
    nc = get_nc()
    in_maps = prepare_in_maps(inputs)
    res = run_bass_kernel_spmd(nc, in_maps, list(range(NCORES)))
    outs = [res.results[c]["out"].reshape(B, Q, H) for c in range(NCORES)]
    return np.concatenate(outs, axis=0)
